# revision 4
# baseline (speedup 1.0000x reference)
"""Distributed KNN online evaluator kernel for 8 trn2 NeuronCores.

Device side (SPMD over 8 cores, bank sharded over N):
  - bank shard (+queries) resident in SBUF bf16, loaded via 7 chained DMAs
  - bf16 matmuls (queries stationary per chunk) -> f32 PSUM,
    1024-col groups on a ring-4 PSUM
  - evacuation in 2048-col units (2 adjacent ring slots, phase-aligned to
    avoid ring wrap) to amortize the ~530ns/instr ACT fixed cost:
      * D-units: DVE tensor_reduce blockmax-4 straight from PSUM
      * A-units: ACT f32->bf16 copy to a 4-region SBUF stage ring; DVE
        folds 2 staged units (4096 elems) with a 2-level TT max tree
        (2x packed bf16) -> blockmax-4
  - DMA out per-(query, block-of-4) maxima as bf16

Host side:
  - adaptive drill-down: select blocks whose blockmax could contain a
    global top-K sim, recompute those sims exactly in f32, take top-K
  - verified: every unselected block provably below the top-K threshold
    (margin covers bf16/matmul fuzz); expands selection until proven
  - class votes with inf weights degenerate to membership -> output is
    [voted classes asc, unvoted classes asc] per query
"""

import numpy as np
import ml_dtypes

import concourse.bass as bass
import concourse.mybir as mybir
from concourse.bass_utils import run_bass_kernel_spmd

BF16 = ml_dtypes.bfloat16

N_CORES = 8
B = 256
D = 128
N_TOTAL = 200000
N_SHARD = N_TOTAL // N_CORES   # 25000
GROUP = 1024                   # psum ring slot cols
N_FULL = 24
TAIL = 512
NCOL = N_FULL * GROUP + TAIL   # 25088
QOFF = 256
N_STEPS_C = N_FULL + 1         # 25 ring slots used per chunk
BLK = 4
SLOTS_C = NCOL // BLK          # 6272 blockmax slots per chunk
K = 200
NUM_CLASSES = 1000
MARGIN = 1.5

PSUM_RING = 4
STAGE_REGIONS = 4              # stage ring: 4 regions x 2048 bf16

# Per-chunk evacuation unit layouts (unit kind, n_slots). 2-slot units
# must not start at ring phase 3 (would wrap the 4-slot PSUM ring).
CHUNK_UNITS = {
    0: [('A', 2), ('A', 2), ('D', 2), ('A', 2), ('A', 2), ('D', 2),
        ('A', 2), ('A', 2), ('D', 2), ('A', 2), ('A', 2), ('A', 2),
        ('T', 1)],
    1: [('D', 1), ('A', 2), ('A', 2), ('D', 2), ('A', 2), ('A', 2),
        ('D', 2), ('A', 2), ('A', 2), ('A', 2), ('A', 2), ('A', 2),
        ('D', 1), ('T', 1)],
}

# bank DMA parts in cols of the packed [D, 256+NCOL] tensor
DMA_COLS = [QOFF + GROUP] + [4 * GROUP] * 5 + [NCOL - 21 * GROUP]

_NC_CACHE = None


def _plan():
    """Static schedule: units (with slots/groups), ACT copies, DVE ops."""
    units = []
    slot = 0
    for c in range(2):
        g = 0
        for kind, ns in CHUNK_UNITS[c]:
            cols = TAIL if kind == 'T' else ns * GROUP
            u = dict(kind=kind, c=c, g0=g, nsl=ns, cols=cols,
                     slot0=slot, idx=len(units))
            if kind == 'A':
                assert ns == 2 and slot % PSUM_RING != 3, (slot, c)
            if kind == 'D' and ns == 2:
                assert slot % PSUM_RING != 3, (slot, c)
            units.append(u)
            g += ns
            slot += ns
        assert g == N_STEPS_C
    # ACT copy order
    n = 0
    for u in units:
        if u['kind'] == 'A':
            u['n'] = n
            n += 1
    # DVE ops: walk units per chunk; directs immediate; fold-pair after
    # every 2nd A-unit; solo fold at chunk end.
    dve_ops = []
    for c in range(2):
        pend = []
        for u in units:
            if u['c'] != c:
                continue
            if u['kind'] in ('D', 'T'):
                dve_ops.append(dict(op='direct', u=u))
            else:
                pend.append(u)
                if len(pend) == 2:
                    dve_ops.append(dict(op='fpair', grp=pend))
                    pend = []
        if pend:
            dve_ops.append(dict(op='fsolo', grp=pend))
    # output units in DVE order with obuf bases
    outs = []
    base = {0: 0, 1: 0}
    nd = nu = 0
    for op in dve_ops:
        if op['op'] == 'direct':
            u = op['u']
            c = u['c']
            nslots = u['cols'] // BLK
            nd += 1
            outs.append(dict(op=op, c=c, base=base[c], nslots=nslots,
                             sem='evacD', cnt=nd))
        else:
            c = op['grp'][0]['c']
            nslots = 1024 if op['op'] == 'fpair' else 512
            nu += 1
            outs.append(dict(op=op, c=c, base=base[c], nslots=nslots,
                             sem='out', cnt=nu))
        op['out'] = outs[-1]
        base[c] += outs[-1]['nslots']
    assert base[0] == SLOTS_C and base[1] == SLOTS_C, base
    return units, dve_ops, outs


def _part_of(g):
    acc = -QOFF
    for pi, cols in enumerate(DMA_COLS):
        acc += cols
        if g * GROUP < acc:
            return pi
    return len(DMA_COLS) - 1


def _build_nc():
    units, dve_ops, outs = _plan()
    nc = bass.Bass("TRN2", target_bir_lowering=False, debug=False,
                   num_devices=N_CORES)
    bankT = nc.dram_tensor("bankT", [D, QOFF + NCOL], mybir.dt.bfloat16,
                           kind="ExternalInput").ap()
    out = nc.dram_tensor("blockmax", [B, SLOTS_C], mybir.dt.bfloat16,
                         kind="ExternalOutput").ap()

    MAX = mybir.AluOpType.max

    # slot -> evac bookkeeping for the tensor ring wait
    evac_of = {}
    nA = nD = 0
    for u in units:
        if u['kind'] == 'A':
            nA += 1
            mark = ('A', nA)
        else:
            nD += 1
            mark = ('D', nD)
        for s in range(u['slot0'], u['slot0'] + u['nsl']):
            evac_of[s] = mark

    with (
        nc.sbuf_tensor([D, QOFF + NCOL], mybir.dt.bfloat16) as banks,
        nc.psum_tensor([128, PSUM_RING * GROUP], mybir.dt.float32) as psum,
        nc.sbuf_tensor([128, STAGE_REGIONS * 2048], mybir.dt.bfloat16
                       ) as stage,
        nc.sbuf_tensor([128, 2048], mybir.dt.bfloat16) as m1,
        nc.sbuf_tensor([128, 2 * SLOTS_C], mybir.dt.bfloat16) as obuf,
        nc.semaphore() as dma_sem,
        nc.semaphore() as dmao_sem,
        nc.semaphore() as mm_sem,
        nc.semaphore() as evacA,
        nc.semaphore() as evacD,
        nc.semaphore() as out_sem,
        nc.semaphore() as stage_free,
        nc.Block() as block,
    ):
        @block.sync
        def _(sync):
            lo = 0
            for cols in DMA_COLS:
                sync.dma_start(banks[:, lo:lo + cols],
                               bankT[:, lo:lo + cols]).then_inc(dma_sem, 16)
                lo += cols
            for o in outs:
                sync.wait_ge(evacD if o['sem'] == 'evacD' else out_sem,
                             o['cnt'])
                c, b0, ns = o['c'], o['base'], o['nslots']
                sync.dma_start(out[c * 128:(c + 1) * 128, b0:b0 + ns],
                               obuf[:, c * SLOTS_C + b0:c * SLOTS_C + b0 + ns]
                               ).then_inc(dmao_sem, 16)

        @block.tensor
        def _(tensor):
            for u in units:
                c = u['c']
                for k in range(u['nsl']):
                    s = u['slot0'] + k
                    g = u['g0'] + k
                    tensor.wait_ge(dma_sem, 16 * (_part_of(g) + 1))
                    if s >= PSUM_RING:
                        sk, cv = evac_of[s - PSUM_RING]
                        tensor.wait_ge(evacA if sk == 'A' else evacD, cv)
                    sl = (s % PSUM_RING) * GROUP
                    cols = TAIL if u['kind'] == 'T' else GROUP
                    nmm = cols // 512
                    for m in range(nmm):
                        mm = tensor.matmul(
                            psum[:, sl + m * 512: sl + (m + 1) * 512],
                            lhsT=banks[:, c * 128:(c + 1) * 128],
                            rhs=banks[:, QOFF + g * GROUP + m * 512:
                                      QOFF + g * GROUP + (m + 1) * 512],
                            start=True, stop=True)
                        if m == nmm - 1:
                            mm.then_inc(mm_sem, 1)

        @block.scalar
        def _(scalar):
            for u in units:
                if u['kind'] != 'A':
                    continue
                n = u['n']
                last_slot = u['slot0'] + u['nsl'] - 1
                if n >= STAGE_REGIONS:
                    scalar.wait_ge(stage_free, n - (STAGE_REGIONS - 1))
                scalar.wait_ge(mm_sem, last_slot + 1)
                sl = (u['slot0'] % PSUM_RING) * GROUP
                ss = (n % STAGE_REGIONS) * 2048
                scalar.copy(stage[:, ss:ss + 2048],
                            psum[:, sl:sl + 2048]).then_inc(evacA, 1)

        @block.vector
        def _(vector):
            for op in dve_ops:
                o = op['out']
                ob = obuf[:, o['c'] * SLOTS_C + o['base']:
                          o['c'] * SLOTS_C + o['base'] + o['nslots']]
                if op['op'] == 'direct':
                    u = op['u']
                    vector.wait_ge(mm_sem, u['slot0'] + u['nsl'])
                    sl = (u['slot0'] % PSUM_RING) * GROUP
                    vector.tensor_reduce(
                        out=ob,
                        in_=psum[:, sl:sl + u['cols']].rearrange(
                            "p (b w) -> p b w", w=BLK),
                        axis=mybir.AxisListType.X,
                        op=MAX,
                    ).then_inc(evacD, 1)
                elif op['op'] == 'fpair':
                    ua, ub = op['grp']
                    vector.wait_ge(evacA, ub['n'] + 1)
                    ra = (ua['n'] % STAGE_REGIONS) * 2048
                    rb = (ub['n'] % STAGE_REGIONS) * 2048
                    vector.tensor_tensor(
                        out=m1[:], in0=stage[:, ra:ra + 2048],
                        in1=stage[:, rb:rb + 2048],
                        op=MAX).then_inc(stage_free, 2)
                    vector.tensor_tensor(out=ob, in0=m1[:, :1024],
                                         in1=m1[:, 1024:],
                                         op=MAX).then_inc(out_sem, 1)
                else:  # fsolo
                    ua, = op['grp']
                    vector.wait_ge(evacA, ua['n'] + 1)
                    ra = (ua['n'] % STAGE_REGIONS) * 2048
                    vector.tensor_tensor(
                        out=m1[:, :1024], in0=stage[:, ra:ra + 1024],
                        in1=stage[:, ra + 1024:ra + 2048],
                        op=MAX).then_inc(stage_free, 1)
                    vector.tensor_tensor(out=ob, in0=m1[:, :512],
                                         in1=m1[:, 512:1024],
                                         op=MAX).then_inc(out_sem, 1)
    return nc


def _get_nc():
    global _NC_CACHE
    if _NC_CACHE is None:
        _NC_CACHE = _build_nc()
    return _NC_CACHE


def _run_device(query_feature, feature_bank, trace=False):
    qT = np.ascontiguousarray(query_feature.astype(np.float32).T
                              ).astype(BF16)
    in_maps = []
    for i in range(N_CORES):
        shard = feature_bank[i * N_SHARD:(i + 1) * N_SHARD].astype(np.float32)
        bt = np.zeros((D, QOFF + NCOL), dtype=BF16)
        bt[:, :QOFF] = qT
        bt[:, QOFF:QOFF + N_SHARD] = np.ascontiguousarray(shard.T
                                                          ).astype(BF16)
        in_maps.append({"bankT": bt})
    nc = _get_nc()
    res = run_bass_kernel_spmd(nc, in_maps, list(range(N_CORES)), trace=trace)
    bm = np.stack([res.results[i]["blockmax"].astype(np.float32)
                   for i in range(N_CORES)])  # [8, 256, SLOTS_C]
    return bm, res


def _slot_rows():
    """[2, SLOTS_C, BLK] local col idx per blockmax slot, per chunk."""
    units, dve_ops, outs = _plan()
    rows = np.empty((2, SLOTS_C, BLK), dtype=np.int64)
    G = GROUP
    for o in outs:
        c, b0, ns = o['c'], o['base'], o['nslots']
        op = o['op']
        if op['op'] == 'direct':
            u = op['u']
            j = np.arange(ns)
            rows[c, b0:b0 + ns] = (u['g0'] * G + BLK * j[:, None]
                                   + np.arange(BLK)[None, :])
        elif op['op'] == 'fpair':
            ua, ub = op['grp']
            ga, gb = ua['g0'], ua['g0'] + 1
            gc, gd = ub['g0'], ub['g0'] + 1
            j = np.arange(1024)
            rows[c, b0:b0 + 1024] = np.stack(
                [ga * G + j, gb * G + j, gc * G + j, gd * G + j], axis=1)
        else:  # fsolo
            ua, = op['grp']
            ga, gb = ua['g0'], ua['g0'] + 1
            j = np.arange(512)
            rows[c, b0:b0 + 512] = np.stack(
                [ga * G + j, ga * G + j + 512,
                 gb * G + j, gb * G + j + 512], axis=1)
    return rows


def _host_topk(bm, query_feature, feature_bank, nsel=192):
    """bm: [8, 256, SLOTS_C] f32 device blockmaxima. Returns top-K indices
    [B, K] into the full bank, matching f32 jax top_k semantics."""
    q = query_feature.astype(np.float32)
    fb = feature_bank.astype(np.float32)
    srows = _slot_rows()  # [2, SLOTS_C, BLK] local cols
    grow_flat = np.empty((2, N_CORES * SLOTS_C, BLK), dtype=np.int64)
    for ch in range(2):
        for cidx in range(N_CORES):
            g = srows[ch] + cidx * N_SHARD
            g[srows[ch] >= N_SHARD] = N_TOTAL  # padding -> sentinel row
            grow_flat[ch, cidx * SLOTS_C:(cidx + 1) * SLOTS_C] = g
    bm_flat = bm.transpose(1, 0, 2).reshape(B, N_CORES * SLOTS_C)
    fb_pad = np.vstack([fb, np.zeros((1, D), np.float32)])

    order = np.argsort(-bm_flat, axis=1)
    sel_sorted = np.take_along_axis(bm_flat, order, axis=1)
    topk_idx = np.empty((B, K), dtype=np.int64)
    pending = np.arange(B)
    nb = nsel
    while len(pending):
        nb = min(nb, bm_flat.shape[1])
        rows = grow_flat[(pending // 128)[:, None],
                         order[pending, :nb]].reshape(len(pending), -1)
        sims = np.einsum("qrd,qd->qr", fb_pad[rows], q[pending],
                         optimize=True)
        sims[rows == N_TOTAL] = -np.inf
        still = []
        for j, b in enumerate(pending):
            o = np.lexsort((rows[j], -sims[j]))[:K]
            tK = sims[j][o[-1]]
            unsel = sel_sorted[b, nb] if nb < bm_flat.shape[1] else -np.inf
            if unsel + MARGIN < tK or nb >= bm_flat.shape[1]:
                topk_idx[b] = rows[j][o]
            else:
                still.append(b)
        pending = np.array(still, dtype=np.int64)
        nb *= 2
    return topk_idx


def _labels_to_output(topk_idx, target_bank):
    tb = np.asarray(target_bank).astype(np.int64)
    out = np.empty((B, NUM_CLASSES), dtype=np.int32)
    allc = np.arange(NUM_CLASSES)
    for b in range(B):
        mask = np.zeros(NUM_CLASSES, dtype=bool)
        mask[tb[topk_idx[b]]] = True
        out[b, :mask.sum()] = allc[mask]
        out[b, mask.sum():] = allc[~mask]
    return out


def kernel(query_feature, feature_bank, target_bank):
    query_feature = np.asarray(query_feature)
    feature_bank = np.asarray(feature_bank)
    target_bank = np.asarray(target_bank)
    bm, _ = _run_device(query_feature, feature_bank)
    topk_idx = _host_topk(bm, query_feature, feature_bank)
    return _labels_to_output(topk_idx, target_bank)


# revision 5
# speedup vs baseline: 1.3579x; 1.3579x over previous
"""Distributed KNN online evaluator kernel for 8 trn2 NeuronCores.

Device side (SPMD over 8 cores, bank sharded over N):
  - bank shard (+queries) resident in SBUF as fp8e4m3, loaded via 7
    chained DMAs (fp8 halves HBM traffic; margin covers the quantization)
  - fp8 matmuls (queries stationary per chunk) -> f32 PSUM,
    1024-col groups on a ring-4 PSUM (fine granularity keeps the
    tensor/ACT/DVE pipeline decoupled; coarser evac units serialize)
  - evacuation split 16 ACT-groups : 8 DVE-groups (+tail) per chunk:
      * D-groups: DVE tensor_reduce blockmax-4 straight from PSUM
      * A-groups: ACT f32->bf16 copy to an 8-slot SBUF stage ring; DVE
        folds 4 staged groups per quad with a 2-level TT max tree
        (2x packed bf16) -> blockmax-4
  - batched DMA out of per-(query, block-of-4) maxima as bf16

Host side:
  - adaptive drill-down: select blocks whose blockmax could contain a
    global top-K sim, recompute those sims exactly in f32, take top-K
  - verified: every unselected block provably below the top-K threshold
    (margin covers fp8/matmul fuzz); expands selection until proven
  - class votes with inf weights degenerate to membership -> output is
    [voted classes asc, unvoted classes asc] per query
"""

import numpy as np
import ml_dtypes

import concourse.bass as bass
import concourse.mybir as mybir
from concourse.bass_utils import run_bass_kernel_spmd

BF16 = ml_dtypes.bfloat16
FP8 = ml_dtypes.float8_e4m3fn

N_CORES = 8
B = 256
D = 128
N_TOTAL = 200000
N_SHARD = N_TOTAL // N_CORES   # 25000
GROUP = 1024
N_FULL = 24
TAIL = 512
NCOL = N_FULL * GROUP + TAIL   # 25088
QOFF = 256
N_STEPS_C = N_FULL + 1
BLK = 4
SLOTS_C = NCOL // BLK          # 6272
K = 200
NUM_CLASSES = 1000
MARGIN = 5.0                   # fp8 inputs: sim err std ~0.6, bound ~5

PSUM_RING = 4
STAGE_RING = 8                 # stage: 8 slots x 1024 bf16
ASSIGN = ['D', 'A', 'A'] * 8   # 16 A + 8 D per chunk (+tail D)
DMA_COLS = [QOFF + GROUP] + [4 * GROUP] * 5 + [NCOL - 21 * GROUP]
OUT_BATCH = 3                  # DVE output units per out-DMA

_NC_CACHE = None


def _plan():
    steps = []
    for c in range(2):
        for g in range(N_STEPS_C):
            kind = 'T' if g == N_FULL else ASSIGN[g]
            steps.append(dict(c=c, g=g, kind=kind, s=len(steps)))
    n = 0
    for st in steps:
        if st['kind'] == 'A':
            st['n'] = n
            n += 1
    dve_ops = []
    for c in range(2):
        pend = []
        for st in steps:
            if st['c'] != c:
                continue
            if st['kind'] in ('D', 'T'):
                dve_ops.append(dict(op='direct', st=st))
            else:
                pend.append(st)
                if len(pend) == 4:
                    dve_ops.append(dict(op='quad', grp=pend))
                    pend = []
        assert not pend
    outs = []
    base = {0: 0, 1: 0}
    nd = nu = 0
    for op in dve_ops:
        if op['op'] == 'direct':
            st = op['st']
            c = st['c']
            nslots = (TAIL if st['kind'] == 'T' else GROUP) // BLK
            nd += 1
            outs.append(dict(op=op, c=c, base=base[c], nslots=nslots,
                             sem='evacD', cnt=nd))
        else:
            c = op['grp'][0]['c']
            nslots = 1024
            nu += 1
            outs.append(dict(op=op, c=c, base=base[c], nslots=nslots,
                             sem='out', cnt=nu))
        op['out'] = outs[-1]
        base[c] += outs[-1]['nslots']
    assert base[0] == SLOTS_C and base[1] == SLOTS_C, base
    # batch consecutive outs (same chunk) into single DMAs
    batches = []
    cur = []
    for o in outs:
        if cur and (len(cur) >= OUT_BATCH or cur[-1]['c'] != o['c']):
            batches.append(cur)
            cur = []
        cur.append(o)
    if cur:
        batches.append(cur)
    return steps, dve_ops, outs, batches


def _part_of(g):
    acc = -QOFF
    for pi, cols in enumerate(DMA_COLS):
        acc += cols
        if g * GROUP < acc:
            return pi
    return len(DMA_COLS) - 1


def _build_nc():
    steps, dve_ops, outs, batches = _plan()
    nc = bass.Bass("TRN2", target_bir_lowering=False, debug=False,
                   num_devices=N_CORES)
    bankT = nc.dram_tensor("bankT", [D, QOFF + NCOL], mybir.dt.float8e4,
                           kind="ExternalInput").ap()
    out = nc.dram_tensor("blockmax", [B, SLOTS_C], mybir.dt.bfloat16,
                         kind="ExternalOutput").ap()

    MAX = mybir.AluOpType.max

    with (
        nc.sbuf_tensor([D, QOFF + NCOL], mybir.dt.float8e4) as banks,
        nc.psum_tensor([128, PSUM_RING * GROUP], mybir.dt.float32) as psum,
        nc.sbuf_tensor([128, STAGE_RING * GROUP], mybir.dt.bfloat16) as stage,
        nc.sbuf_tensor([128, 2048], mybir.dt.bfloat16) as m1,
        nc.sbuf_tensor([128, 2 * SLOTS_C], mybir.dt.bfloat16) as obuf,
        nc.semaphore() as dma_sem,
        nc.semaphore() as dmao_sem,
        nc.semaphore() as mm_sem,
        nc.semaphore() as evacA,
        nc.semaphore() as evacD,
        nc.semaphore() as out_sem,
        nc.semaphore() as stage_free,
        nc.Block() as block,
    ):
        @block.sync
        def _(sync):
            lo = 0
            for cols in DMA_COLS:
                sync.dma_start(banks[:, lo:lo + cols],
                               bankT[:, lo:lo + cols]).then_inc(dma_sem, 16)
                lo += cols
            for batch in batches:
                need_d = max([o['cnt'] for o in batch
                              if o['sem'] == 'evacD'], default=0)
                need_u = max([o['cnt'] for o in batch
                              if o['sem'] == 'out'], default=0)
                if need_d:
                    sync.wait_ge(evacD, need_d)
                if need_u:
                    sync.wait_ge(out_sem, need_u)
                c = batch[0]['c']
                b0 = batch[0]['base']
                ns = sum(o['nslots'] for o in batch)
                sync.dma_start(out[c * 128:(c + 1) * 128, b0:b0 + ns],
                               obuf[:, c * SLOTS_C + b0:c * SLOTS_C + b0 + ns]
                               ).then_inc(dmao_sem, 16)

        @block.tensor
        def _(tensor):
            nA = nD = 0
            evac_of = {}
            for st in steps:
                s, c, g, kind = st['s'], st['c'], st['g'], st['kind']
                tensor.wait_ge(dma_sem, 16 * (_part_of(g) + 1))
                if s >= PSUM_RING:
                    sk, cv = evac_of[s - PSUM_RING]
                    tensor.wait_ge(evacA if sk == 'A' else evacD, cv)
                if kind == 'A':
                    nA += 1
                    evac_of[s] = ('A', nA)
                else:
                    nD += 1
                    evac_of[s] = ('D', nD)
                sl = (s % PSUM_RING) * GROUP
                cols = TAIL if kind == 'T' else GROUP
                nmm = cols // 512
                for m in range(nmm):
                    mm = tensor.matmul(
                        psum[:, sl + m * 512: sl + (m + 1) * 512],
                        lhsT=banks[:, c * 128:(c + 1) * 128],
                        rhs=banks[:, QOFF + g * GROUP + m * 512:
                                  QOFF + g * GROUP + (m + 1) * 512],
                        start=True, stop=True)
                    if m == nmm - 1:
                        mm.then_inc(mm_sem, 1)

        @block.scalar
        def _(scalar):
            for st in steps:
                if st['kind'] != 'A':
                    continue
                s, n = st['s'], st['n']
                if n >= STAGE_RING:
                    scalar.wait_ge(stage_free, n - (STAGE_RING - 1))
                scalar.wait_ge(mm_sem, s + 1)
                sl = (s % PSUM_RING) * GROUP
                ss = (n % STAGE_RING) * GROUP
                scalar.copy(stage[:, ss:ss + GROUP],
                            psum[:, sl:sl + GROUP]).then_inc(evacA, 1)

        @block.vector
        def _(vector):
            for op in dve_ops:
                o = op['out']
                ob = obuf[:, o['c'] * SLOTS_C + o['base']:
                          o['c'] * SLOTS_C + o['base'] + o['nslots']]
                if op['op'] == 'direct':
                    st = op['st']
                    vector.wait_ge(mm_sem, st['s'] + 1)
                    sl = (st['s'] % PSUM_RING) * GROUP
                    cols = TAIL if st['kind'] == 'T' else GROUP
                    vector.tensor_reduce(
                        out=ob,
                        in_=psum[:, sl:sl + cols].rearrange(
                            "p (b w) -> p b w", w=BLK),
                        axis=mybir.AxisListType.X,
                        op=MAX,
                    ).then_inc(evacD, 1)
                else:  # quad: 4 staged groups -> 1024 block-4 maxima
                    ns = [g['n'] for g in op['grp']]
                    vector.wait_ge(evacA, ns[-1] + 1)
                    ra = (ns[0] % STAGE_RING) * GROUP
                    rb = (ns[2] % STAGE_RING) * GROUP
                    vector.tensor_tensor(
                        out=m1[:], in0=stage[:, ra:ra + 2048],
                        in1=stage[:, rb:rb + 2048],
                        op=MAX).then_inc(stage_free, 4)
                    vector.tensor_tensor(out=ob, in0=m1[:, :1024],
                                         in1=m1[:, 1024:],
                                         op=MAX).then_inc(out_sem, 1)
    return nc


def _get_nc():
    global _NC_CACHE
    if _NC_CACHE is None:
        _NC_CACHE = _build_nc()
    return _NC_CACHE


def _run_device(query_feature, feature_bank, trace=False):
    qT = np.ascontiguousarray(query_feature.astype(np.float32).T
                              ).astype(FP8)
    in_maps = []
    for i in range(N_CORES):
        shard = feature_bank[i * N_SHARD:(i + 1) * N_SHARD].astype(np.float32)
        bt = np.zeros((D, QOFF + NCOL), dtype=FP8)
        bt[:, :QOFF] = qT
        bt[:, QOFF:QOFF + N_SHARD] = np.ascontiguousarray(shard.T
                                                          ).astype(FP8)
        in_maps.append({"bankT": bt})
    nc = _get_nc()
    res = run_bass_kernel_spmd(nc, in_maps, list(range(N_CORES)), trace=trace)
    bm = np.stack([res.results[i]["blockmax"].astype(np.float32)
                   for i in range(N_CORES)])  # [8, 256, SLOTS_C]
    return bm, res


def _slot_rows():
    """[SLOTS_C, BLK] local col idx per blockmax slot (same both chunks)."""
    _, dve_ops, outs, _ = _plan()
    rows = np.empty((SLOTS_C, BLK), dtype=np.int64)
    G = GROUP
    for o in outs:
        if o['c'] != 0:
            continue
        b0, ns = o['base'], o['nslots']
        op = o['op']
        if op['op'] == 'direct':
            st = op['st']
            j = np.arange(ns)
            rows[b0:b0 + ns] = (st['g'] * G + BLK * j[:, None]
                                + np.arange(BLK)[None, :])
        else:
            ga, gb, gc, gd = [x['g'] for x in op['grp']]
            j = np.arange(1024)
            rows[b0:b0 + 1024] = np.stack(
                [ga * G + j, gb * G + j, gc * G + j, gd * G + j], axis=1)
    return rows


def _host_topk(bm, query_feature, feature_bank, nsel=256):
    """bm: [8, 256, SLOTS_C] f32 device blockmaxima. Returns top-K indices
    [B, K] into the full bank, matching f32 jax top_k semantics."""
    q = query_feature.astype(np.float32)
    fb = feature_bank.astype(np.float32)
    srows = _slot_rows()
    grow_flat = np.empty((N_CORES * SLOTS_C, BLK), dtype=np.int64)
    for cidx in range(N_CORES):
        g = srows + cidx * N_SHARD
        g[srows >= N_SHARD] = N_TOTAL
        grow_flat[cidx * SLOTS_C:(cidx + 1) * SLOTS_C] = g
    bm_flat = bm.transpose(1, 0, 2).reshape(B, N_CORES * SLOTS_C)
    fb_pad = np.vstack([fb, np.zeros((1, D), np.float32)])

    order = np.argsort(-bm_flat, axis=1)
    sel_sorted = np.take_along_axis(bm_flat, order, axis=1)
    topk_idx = np.empty((B, K), dtype=np.int64)
    pending = np.arange(B)
    nb = nsel
    while len(pending):
        nb = min(nb, bm_flat.shape[1])
        rows = grow_flat[order[pending, :nb]].reshape(len(pending), -1)
        sims = np.einsum("qrd,qd->qr", fb_pad[rows], q[pending],
                         optimize=True)
        sims[rows == N_TOTAL] = -np.inf
        still = []
        for j, b in enumerate(pending):
            o = np.lexsort((rows[j], -sims[j]))[:K]
            tK = sims[j][o[-1]]
            unsel = sel_sorted[b, nb] if nb < bm_flat.shape[1] else -np.inf
            if unsel + MARGIN < tK or nb >= bm_flat.shape[1]:
                topk_idx[b] = rows[j][o]
            else:
                still.append(b)
        pending = np.array(still, dtype=np.int64)
        nb *= 2
    return topk_idx


def _labels_to_output(topk_idx, target_bank):
    tb = np.asarray(target_bank).astype(np.int64)
    out = np.empty((B, NUM_CLASSES), dtype=np.int32)
    allc = np.arange(NUM_CLASSES)
    for b in range(B):
        mask = np.zeros(NUM_CLASSES, dtype=bool)
        mask[tb[topk_idx[b]]] = True
        out[b, :mask.sum()] = allc[mask]
        out[b, mask.sum():] = allc[~mask]
    return out


def kernel(query_feature, feature_bank, target_bank):
    query_feature = np.asarray(query_feature)
    feature_bank = np.asarray(feature_bank)
    target_bank = np.asarray(target_bank)
    bm, _ = _run_device(query_feature, feature_bank)
    topk_idx = _host_topk(bm, query_feature, feature_bank)
    return _labels_to_output(topk_idx, target_bank)
